# revision 50
# baseline (speedup 1.0000x reference)
"""Trainium2 Bass kernel for nn_Attention_76321568850409 (channel attention).

Data-parallel over batch: 8 samples -> 8 NeuronCores, params replicated.
Per-core pipeline (sample s, c=192, n=128*128), layout [c, n] bf16 compute:
  V0 = conv1x1(x)+b      [PE fp32 -> bf16]
  v  = dwconv3x3(V0)+db  [DVE scalar_tensor_tensor taps]
  v_t = transpose(v)     [PE] (unscaled; l2 norm folded in later as scales)
  Q0 = conv1x1(edge)+b; q = dwconv(Q0); ||q||; G_qv = q @ v^T  [PE grams]
  K0 = conv1x1(grad)+b; k = dwconv(K0); ||k||; G_kv = k @ v^T
  attn1 = softmax(G_qv * rq*t row-scale * rv col-scale)  (per 24x24 head block)
  attn2 = likewise;  A^T = attn2^T @ attn1^T per head
  vtilde = rv * transpose(v_t)   (rv folded into evac scale)
  out = proj(A @ vtilde) + b_p   [PE] -> fp32 DMA out
"""

import numpy as np
import os
from contextlib import ExitStack
_SKIP = set(os.environ.get("K_SKIP", "").split(","))

B, C, H, W = 8, 192, 128, 128
HEADS = 8
CPH = C // HEADS          # 24
N = H * W                 # 16384
G = 2                     # head groups (4 heads = 96 chans each)
CG = 96
NT = 512

_COMPILED = {}


def _build(nc):
    import concourse.bass as bass
    import concourse.mybir as mybir
    from concourse import tile

    f32 = mybir.dt.float32
    bf16 = mybir.dt.bfloat16
    AX = mybir.AxisListType
    OP = mybir.AluOpType
    AF = mybir.ActivationFunctionType

    x_d = nc.declare_dram_parameter("x", [C, N], f32, isOutput=False)
    e_d = nc.declare_dram_parameter("edge", [C, N], f32, isOutput=False)
    g_d = nc.declare_dram_parameter("grad", [C, N], f32, isOutput=False)
    wT_q = nc.declare_dram_parameter("wT_q", [CG, 2 * C], f32, isOutput=False)
    wT_k = nc.declare_dram_parameter("wT_k", [CG, 2 * C], f32, isOutput=False)
    wT_v = nc.declare_dram_parameter("wT_v", [CG, 2 * C], f32, isOutput=False)
    wT_p = nc.declare_dram_parameter("wT_p", [CG, 2 * C], f32, isOutput=False)
    # packed small params [C, 16]: 0..8 dw taps (t=(dy+1)*3+dx+1), 9 conv bias,
    # 10 dw bias, 11 temperature, 12 proj bias
    sm_q = nc.declare_dram_parameter("sm_q", [128, 32], f32, isOutput=False)
    sm_k = nc.declare_dram_parameter("sm_k", [128, 32], f32, isOutput=False)
    sm_v = nc.declare_dram_parameter("sm_v", [128, 32], f32, isOutput=False)
    ident = nc.declare_dram_parameter("ident", [128, 128], f32, isOutput=False)
    mask_d = nc.declare_dram_parameter("mask", [CG, CG], f32, isOutput=False)
    out_d = nc.declare_dram_parameter("out", [C, N], f32, isOutput=True)

    CH = [(0, 128), (128, 64)]   # [c,n] channel chunks

    with tile.TileContext(nc) as tc, ExitStack() as ctx:
        pw = ctx.enter_context(tc.tile_pool(name="weights", bufs=1))
        pin = ctx.enter_context(tc.tile_pool(name="inslab", bufs=2))
        pU = ctx.enter_context(tc.tile_pool(name="bigU", bufs=1))
        pV = ctx.enter_context(tc.tile_pool(name="bigV", bufs=4))
        pvt = ctx.enter_context(tc.tile_pool(name="vtp", bufs=1))
        pst = ctx.enter_context(tc.tile_pool(name="stage", bufs=4))
        psm = ctx.enter_context(tc.tile_pool(name="small", bufs=1))
        pout = ctx.enter_context(tc.tile_pool(name="oslab", bufs=2))
        pps = ctx.enter_context(tc.tile_pool(name="ps", bufs=2, space="PSUM"))
        pga = ctx.enter_context(tc.tile_pool(name="ga", bufs=1, space="PSUM"))
        pp1 = ctx.enter_context(tc.tile_pool(name="ps1", bufs=2, space="PSUM"))
        pp2 = ctx.enter_context(tc.tile_pool(name="ps2", bufs=1, space="PSUM"))

        # ---------------- weights/constants ----------------
        wq = pw.tile([CG, 2 * C], f32, tag="wq")
        wk = pw.tile([CG, 2 * C], f32, tag="wk")
        wv = pw.tile([CG, 2 * C], f32, tag="wv")
        for wt, src in ((wq, wT_q), (wk, wT_k), (wv, wT_v)):
            nc.sync.dma_start(wt[:], src[:])
        wpf = pw.tile([CG, 2 * C], f32, tag="wpf")
        nc.sync.dma_start(wpf[:], wT_p[:])
        wp = pw.tile([CG, 2 * C], bf16, tag="wp")
        nc.vector.tensor_copy(wp[:], wpf[:])

        smq = pw.tile([128, 32], f32, tag="smq")
        smk = pw.tile([128, 32], f32, tag="smk")
        smv = pw.tile([128, 32], f32, tag="smv")
        for st_, src in ((smq, sm_q), (smk, sm_k), (smv, sm_v)):
            nc.sync.dma_start(st_[:], src[:])
        idf = pw.tile([128, 128], f32, tag="idf")
        nc.sync.dma_start(idf[:], ident[:])
        maskt = pw.tile([CG, CG], f32, tag="maskt")
        nc.sync.dma_start(maskt[:], mask_d[:])
        idb = pw.tile([128, 128], bf16, tag="idb")
        nc.vector.tensor_copy(idb[:], idf[:])

        # ---------------- big buffers ----------------
        U0a = pU.tile([128, N], bf16, tag="U0a")      # dwconv input chans 0:128
        U0b = pU.tile([64, N], bf16, tag="U0b")       # chans 128:192

        vt = pvt.tile([128, 128 * C], bf16, tag="vt")  # [n,c]: chunk cc at cols cc*C

        nsq = psm.tile([128, 48], f32, tag="nsq")   # ||.||^2: cols (q0,q1,k0,k1,v0,v1)+6*half
        nrm = psm.tile([128, 6], f32, tag="nrm")    # 1/||.||: q0,q1,k0,k1,v0,v1 (t folded for q/k)

        def conv1x1(src_d, wtile, sm):
            SL = 1024
            for s in range(N // SL):
                i0 = pin.tile([CG, SL], f32, tag="in0")
                i1 = pin.tile([CG, SL], f32, tag="in1")
                nc.sync.dma_start(i0[:], src_d[0:CG, s * SL:(s + 1) * SL])
                nc.sync.dma_start(i1[:], src_d[CG:C, s * SL:(s + 1) * SL])
                for t in range(SL // NT):
                    n0 = s * SL + t * NT
                    ps_a = pps.tile([128, NT], f32, tag="psA")
                    ps_b = pps.tile([64, NT], f32, tag="psB")
                    for ki, it in ((0, i0), (1, i1)):
                        nc.tensor.matmul(ps_a[:], wtile[:, ki * C:ki * C + 128],
                                         it[:, t * NT:(t + 1) * NT],
                                         start=(ki == 0), stop=(ki == 1))
                        nc.tensor.matmul(ps_b[:], wtile[:, ki * C + 128:ki * C + 192],
                                         it[:, t * NT:(t + 1) * NT],
                                         start=(ki == 0), stop=(ki == 1))
                    nc.vector.tensor_scalar(U0a[:, n0:n0 + NT], ps_a[:],
                                            sm[:, 9:10], None, OP.add)
                    nc.scalar.activation(U0b[:, n0:n0 + NT], ps_b[:],
                                         AF.Identity, bias=sm[0:64, 25:26],
                                         scale=1.0)

        QR = 16   # rows per part
        NQ = H // QR

        def dwconv_part(sm, part, vha, vhb):
            if "taps" in _SKIP:
                for ci in range(2):
                    y = (vha if ci == 0 else vhb)
                    nc.vector.memset(y[:, 0:4], 0.0)
                return
            """(vha,vhb) <- dwconv3x3(U0)+db for image rows [64*half, 64*half+64).

            Engine split: see act_taps/gps_taps."""
            r0 = QR * part
            for ci, (cb, cw) in enumerate(CH):
                U3 = (U0a if ci == 0 else U0b)[:, :].rearrange("p (h w) -> p h w", h=H)
                y3 = (vha if ci == 0 else vhb)[:, :].rearrange("p (h w) -> p h w", h=QR)
                wcol = lambda t: sm[0:cw, 16 * ci + t:16 * ci + t + 1]
                nc.vector.tensor_scalar(y3[:, :, :], U3[:, r0:r0 + QR, :],
                                        wcol(4), wcol(10), OP.mult, OP.add)
                act_taps = [(-1, -1), (-1, 1), (0, -1), (0, 1), (1, -1)]
                gps_taps = [(-1, 0), (1, 0)]
                for dy in (-1, 0, 1):
                    for dx in (-1, 0, 1):
                        if dy == 0 and dx == 0:
                            continue
                        t = (dy + 1) * 3 + (dx + 1)
                        sy0, sy1 = max(0, r0 + dy), min(H, r0 + QR + dy)
                        oy0, oy1 = sy0 - dy - r0, sy1 - dy - r0
                        sx0, sx1 = max(0, dx), min(W, W + dx)
                        ox0, ox1 = sx0 - dx, sx1 - dx
                        if (dy, dx) in act_taps:
                            # ACT: tmp = w*U (1x, alignment-immune), DVE: y += tmp
                            nh = oy1 - oy0
                            step = 16
                            for h0 in range(0, nh, step):
                                h1 = min(nh, h0 + step)
                                tmp = pst.tile([128, step * W], bf16, tag="acttmp",
                                               name=f"tmp{part}{ci}{t}{h0}")
                                t3 = tmp[0:cw, 0:(h1 - h0) * W].rearrange(
                                    "p (h w) -> p h w", h=h1 - h0)
                                nc.scalar.activation(
                                    t3[:, :, 0:sx1 - sx0],
                                    U3[:, sy0 + h0:sy0 + (h1 - h0) + h0,
                                       sx0:sx1],
                                    AF.Copy, bias=0.0, scale=wcol(t))
                                add_eng = (nc.gpsimd if (dy, dx) in
                                           ((-1, -1), (-1, 1), (1, -1)) else nc.vector)
                                add_eng.tensor_tensor(
                                    y3[:, oy0 + h0:oy0 + h1, ox0:ox1],
                                    y3[:, oy0 + h0:oy0 + h1, ox0:ox1],
                                    t3[:, :, 0:sx1 - sx0], OP.add)
                        elif (dy, dx) in gps_taps:
                            nc.vector.scalar_tensor_tensor(
                                y3[:, oy0:oy1, ox0:ox1], U3[:, sy0:sy1, sx0:sx1],
                                wcol(t), y3[:, oy0:oy1, ox0:ox1], OP.mult, OP.add)
                        else:
                            nc.vector.scalar_tensor_tensor(
                                y3[:, oy0:oy1, ox0:ox1], U3[:, sy0:sy1, sx0:sx1],
                                wcol(t), y3[:, oy0:oy1, ox0:ox1], OP.mult, OP.add)

        def squares(part, base, vha, vhb):
            """in-place Square; accumulate into nsq col base*8 + 2*part + ci."""
            for ci, (cb, cw) in enumerate(CH):
                yy = (vha if ci == 0 else vhb)
                col = base * 16 + 2 * part + ci
                nc.scalar.activation(yy[:, :], yy[:, :], AF.Square,
                                     accum_out=nsq[0:cw, col:col + 1])

        def transpose_grams(part, vha, vhb, gacc=None, first=False, last=False):
            for cc in range(QR * W // 128):
                n0 = cc * 128
                chunk = part * (QR * W // 128) + cc
                pt = pp1.tile([128, C], bf16, tag="psS")
                nc.tensor.transpose(pt[:, 0:128], vha[:, n0:n0 + 128], idb[:])
                nc.tensor.transpose(pt[:, 128:C], vhb[:, n0:n0 + 128], idb[0:64, 0:64])
                if gacc is None:
                    nc.vector.tensor_copy(vt[:, chunk * C:(chunk + 1) * C], pt[:])
                else:
                    st = pst.tile([128, C], bf16, tag="tstage")
                    nc.vector.tensor_copy(st[:], pt[:])
                    for gi in range(G):
                        nc.tensor.matmul(
                            gacc[gi], st[:, gi * CG:(gi + 1) * CG],
                            vt[:, chunk * C + gi * CG:chunk * C + (gi + 1) * CG],
                            start=(first and cc == 0 and gi == 0),
                            stop=(last and cc == QR * W // 128 - 1 and gi == 1))

        # ================= V phase =================
        conv1x1(x_d, wv, smv)
        for part in range(NQ):
            vha = pV.tile([128, QR * W], bf16, tag="vha", name=f"v{part}a")
            vhb = pV.tile([64, QR * W], bf16, tag="vhb", name=f"v{part}b")
            dwconv_part(smv, part, vha, vhb)
            transpose_grams(part, vha, vhb)      # fills vt
            squares(part, 2, vha, vhb)

        # ================= Q phase =================
        conv1x1(e_d, wq, smq)
        gq0 = pga.tile([CG, 2 * CG], f32, tag="g", name="gq0")
        gq = [gq0[:, 0:CG], gq0[:, CG:2 * CG]]
        for part in range(NQ):
            vha = pV.tile([128, QR * W], bf16, tag="vha", name=f"q{part}a")
            vhb = pV.tile([64, QR * W], bf16, tag="vhb", name=f"q{part}b")
            dwconv_part(smq, part, vha, vhb)
            transpose_grams(part, vha, vhb, gacc=gq, first=(part == 0),
                            last=(part == NQ - 1))
            squares(part, 0, vha, vhb)
        gq_sb = psm.tile([CG, 2 * CG], f32, tag="gqsb")
        nc.vector.tensor_copy(gq_sb[:], gq0[:])

        # ================= K phase =================
        conv1x1(g_d, wk, smk)
        gk0 = pga.tile([CG, 2 * CG], f32, tag="g", name="gk0")
        gk = [gk0[:, 0:CG], gk0[:, CG:2 * CG]]
        for part in range(NQ):
            vha = pV.tile([128, QR * W], bf16, tag="vha", name=f"k{part}a")
            vhb = pV.tile([64, QR * W], bf16, tag="vhb", name=f"k{part}b")
            dwconv_part(smk, part, vha, vhb)
            transpose_grams(part, vha, vhb, gacc=gk, first=(part == 0),
                            last=(part == NQ - 1))
            squares(part, 1, vha, vhb)
        gk_sb = psm.tile([CG, 2 * CG], f32, tag="gksb")
        nc.vector.tensor_copy(gk_sb[:], gk0[:])

        # ============== norms -> inverse scales ==============
        tots = psm.tile([128, 6], f32, tag="tots")
        for b in range(3):
            o = b * 16
            nc.vector.tensor_tensor(tots[:, 2 * b:2 * b + 2], nsq[:, o:o + 2],
                                    nsq[:, o + 2:o + 4], OP.add)
            for p_ in range(2, NQ):
                nc.vector.tensor_tensor(tots[:, 2 * b:2 * b + 2],
                                        tots[:, 2 * b:2 * b + 2],
                                        nsq[:, o + 2 * p_:o + 2 * p_ + 2], OP.add)
        rt = psm.tile([128, 6], f32, tag="rt")
        nc.scalar.activation(rt[:, :], tots[:, :], AF.Sqrt)
        nc.vector.reciprocal(nrm[:, :], rt[:, :])
        # fold temperature into q/k inverse-norms
        for ci in range(2):
            for col, sm in ((0, smq), (2, smk)):
                cw = 128 if ci == 0 else 64
                nc.vector.tensor_tensor(
                    nrm[0:cw, col + ci:col + ci + 1],
                    nrm[0:cw, col + ci:col + ci + 1],
                    sm[0:cw, 16 * ci + 11:16 * ci + 12], OP.mult)

        # rv as broadcast matrix RVb[g][c, d] = rv[g*96+d] via K=1 matmul
        rvrow_ps = pp1.tile([1, C], f32, tag="psS")
        nc.tensor.transpose(rvrow_ps[:, 0:128], nrm[0:128, 4:5], idf[:])
        nc.tensor.transpose(rvrow_ps[:, 128:C], nrm[0:64, 5:6], idf[0:64, 0:64])
        rvrow = psm.tile([1, C], f32, tag="rvrow_sb")
        nc.vector.tensor_copy(rvrow[:], rvrow_ps[:])
        ones1 = psm.tile([1, CG], f32, tag="ones1")
        nc.vector.memset(ones1[:], 1.0)
        RVb = psm.tile([CG, 2 * CG], f32, tag="RVb")
        for gi in range(G):
            rb = pp1.tile([CG, CG], f32, tag="psS")
            nc.tensor.matmul(rb[:], ones1[:], rvrow[:, gi * CG:(gi + 1) * CG],
                             start=True, stop=True)
            nc.vector.tensor_copy(RVb[:, gi * CG:(gi + 1) * CG], rb[:])

        # group-layout row scales (group rows 96 = chunk rows 0:96 | 96:128+0:64)
        def group_scale(qcol, tag):
            sc = psm.tile([CG, 2], f32, tag=tag)
            nc.vector.tensor_copy(sc[0:CG, 0:1], nrm[0:CG, qcol:qcol + 1])
            nc.sync.dma_start(sc[0:32, 1:2], nrm[96:128, qcol:qcol + 1])
            nc.sync.dma_start(sc[32:CG, 1:2], nrm[0:64, qcol + 1:qcol + 2])
            return sc

        def softmax(g_sb, qcol, tagp):
            """returns blockdiag attn [96, 2*96] bf16 (group gi at cols gi*96).
            Softmax over full 96 cols with -inf mask off the 24x24 diag blocks."""
            sc = group_scale(qcol, "sc" + tagp)
            Ablk = psm.tile([CG, G * CG], bf16, tag="Ablk" + tagp)
            for gi in range(G):
                gsl = g_sb[:, gi * CG:(gi + 1) * CG]
                nc.vector.tensor_scalar(gsl, gsl, sc[:, gi:gi + 1], None, OP.mult)
                nc.vector.tensor_tensor(gsl, gsl, RVb[:, gi * CG:(gi + 1) * CG], OP.mult)
                nc.vector.tensor_tensor(gsl, gsl, maskt[:], OP.add)
                mx = psm.tile([CG, 1], f32, tag="mx" + tagp)
                nc.vector.tensor_reduce(mx[:], gsl, AX.X, OP.max)
                nc.vector.tensor_scalar(gsl, gsl, mx[:], None, OP.subtract)
                se = psm.tile([CG, 1], f32, tag="se" + tagp)
                nc.scalar.activation(gsl, gsl, AF.Exp, accum_out=se[:])
                nc.vector.reciprocal(se[:], se[:])
                nc.vector.tensor_scalar(Ablk[:, gi * CG:(gi + 1) * CG], gsl,
                                        se[:], None, OP.mult)
            return Ablk

        a1B = softmax(gq_sb, 0, "q")   # attn1 blockdiag [c, d] per group
        a2B = softmax(gk_sb, 2, "k")   # attn2 blockdiag [d, e] per group

        # A^T_group = attn2^T @ attn1^T = a2B^T @ transpose(a1B)  (one MM per group)
        AT_sb = psm.tile([CG, 2 * CG], bf16, tag="ATsb")
        for gi in range(G):
            a1T_ps = pp1.tile([CG, CG], bf16, tag="psS")
            nc.tensor.transpose(a1T_ps[:], a1B[:, gi * CG:(gi + 1) * CG],
                                idb[0:CG, 0:CG])
            a1T = pst.tile([CG, CG], bf16, tag="a1T")
            nc.vector.tensor_copy(a1T[:], a1T_ps[:])
            at_ps = pp1.tile([CG, CG], f32, tag="psS")
            nc.tensor.matmul(at_ps[:], a2B[:, gi * CG:(gi + 1) * CG], a1T[:],
                             start=True, stop=True)
            nc.vector.tensor_copy(AT_sb[:, gi * CG:(gi + 1) * CG], at_ps[:])

        # chunk-layout A^T for attn@v: ATa [128, 192], ATb [64, 192]
        ATa = psm.tile([128, C], bf16, tag="ATa")
        ATb = psm.tile([64, C], bf16, tag="ATb")
        nc.vector.memset(ATa[:], 0.0)
        nc.vector.memset(ATb[:], 0.0)
        nc.vector.tensor_copy(ATa[0:CG, 0:CG], AT_sb[:, 0:CG])
        nc.sync.dma_start(ATa[96:128, CG:2 * CG], AT_sb[0:32, CG:2 * CG])
        nc.sync.dma_start(ATb[0:64, CG:2 * CG], AT_sb[32:CG, CG:2 * CG])

        # ======== vtilde (rv folded) into U0 slots ========
        vta = pU.tile([128, N], bf16, tag="U0a")
        vtb = pU.tile([64, N], bf16, tag="U0b")
        for cc in range(128):
            tb = pp1.tile([128, 128], bf16, tag="psS", name=f"vr{cc}")
            nc.tensor.transpose(tb[:], vt[:, cc * C:cc * C + 128], idb[:])
            nc.vector.tensor_scalar(vta[:, cc * 128:(cc + 1) * 128], tb[:],
                                    nrm[:, 4:5], None, OP.mult)
            tb2 = pp1.tile([64, 128], bf16, tag="psS", name=f"vs{cc}")
            nc.tensor.transpose(tb2[:], vt[:, cc * C + 128:cc * C + C], idb[:])
            nc.scalar.activation(vtb[:, cc * 128:(cc + 1) * 128], tb2[:],
                                 AF.Copy, bias=0.0, scale=nrm[0:64, 5:6])

        # ============ attn@v + proj ============
        SL = 512
        for s in range(N // SL):
            aoa = pout.tile([CG, SL], bf16, tag="aoa")
            aob = pout.tile([CG, SL], bf16, tag="aob")
            for t in range(SL // NT):
                n0 = s * SL + t * NT
                pa = pps.tile([CG, NT], f32, tag="psB")
                pb = pp2.tile([CG, NT], f32, tag="psB2")
                nc.tensor.matmul(pa[:], ATa[:, 0:CG], vta[:, n0:n0 + NT],
                                 start=True, stop=False)
                nc.tensor.matmul(pa[:], ATb[:, 0:CG], vtb[:, n0:n0 + NT],
                                 start=False, stop=True)
                nc.tensor.matmul(pb[:], ATa[:, CG:C], vta[:, n0:n0 + NT],
                                 start=True, stop=False)
                nc.tensor.matmul(pb[:], ATb[:, CG:C], vtb[:, n0:n0 + NT],
                                 start=False, stop=True)
                nc.vector.tensor_copy(aoa[:, t * NT:(t + 1) * NT], pa[:])
                nc.scalar.copy(aob[:, t * NT:(t + 1) * NT], pb[:])
            ot = pout.tile([128, SL], f32, tag="ot")
            ob = pout.tile([64, SL], f32, tag="ob")
            for t in range(SL // NT):
                n0 = t * NT
                pc_ = pps.tile([128, NT], f32, tag="psA")
                pd_ = pp2.tile([64, NT], f32, tag="psB2")
                nc.tensor.matmul(pc_[:], wp[:, 0:128], aoa[:, n0:n0 + NT],
                                 start=True, stop=False)
                nc.tensor.matmul(pc_[:], wp[:, C:C + 128], aob[:, n0:n0 + NT],
                                 start=False, stop=True)
                nc.tensor.matmul(pd_[:], wp[:, 128:C], aoa[:, n0:n0 + NT],
                                 start=True, stop=False)
                nc.tensor.matmul(pd_[:], wp[:, C + 128:C + C], aob[:, n0:n0 + NT],
                                 start=False, stop=True)
                nc.vector.tensor_scalar(ot[:, n0:n0 + NT], pc_[:],
                                        smv[:, 12:13], None, OP.add)
                nc.scalar.activation(ob[:, n0:n0 + NT], pd_[:], AF.Identity,
                                     bias=smv[0:64, 28:29], scale=1.0)
            nc.sync.dma_start(out_d[0:128, s * SL:(s + 1) * SL], ot[:])
            nc.sync.dma_start(out_d[128:C, s * SL:(s + 1) * SL], ob[:])

    return nc


def _blockmask():
    m = np.full((CG, CG), -1e30, np.float32)
    for hh in range(4):
        m[hh * CPH:(hh + 1) * CPH, hh * CPH:(hh + 1) * CPH] = 0.0
    return m


def _host_prep(inputs):
    x = np.asarray(inputs["x"], np.float32)
    edge = np.asarray(inputs["edge"], np.float32)
    grad = np.asarray(inputs["grad_in"], np.float32)
    temp = np.asarray(inputs["temperature"], np.float32).reshape(HEADS)
    t_chan = np.repeat(temp, CPH).astype(np.float32)

    def smpack(dw, db, bconv, bproj=None):
        sm = np.zeros((C, 16), np.float32)
        sm[:, 0:9] = np.asarray(dw, np.float32).reshape(C, 9)
        sm[:, 9] = np.asarray(bconv, np.float32)
        sm[:, 10] = np.asarray(db, np.float32)
        sm[:, 11] = t_chan
        if bproj is not None:
            sm[:, 12] = np.asarray(bproj, np.float32)
        out = np.zeros((128, 32), np.float32)
        out[:, 0:16] = sm[0:128]
        out[0:64, 16:32] = sm[128:C]
        return out

    def wpack(w):
        wt = np.asarray(w, np.float32).T   # [ci, co]
        return np.ascontiguousarray(np.concatenate([wt[0:CG], wt[CG:C]], axis=1))

    shared = {
        "wT_q": wpack(inputs["w_q"]),
        "wT_k": wpack(inputs["w_k"]),
        "wT_v": wpack(inputs["w_v"]),
        "wT_p": wpack(inputs["w_proj"]),
        "sm_q": smpack(inputs["dw_q"], inputs["db_q"], inputs["b_q"]),
        "sm_k": smpack(inputs["dw_k"], inputs["db_k"], inputs["b_k"]),
        "sm_v": smpack(inputs["dw_v"], inputs["db_v"], inputs["b_v"],
                       inputs["b_proj"]),
        "ident": np.eye(128, dtype=np.float32),
        "mask": _blockmask(),
    }
    maps = []
    for i in range(B):
        m = dict(shared)
        m["x"] = np.ascontiguousarray(x[i].reshape(C, N))
        m["edge"] = np.ascontiguousarray(edge[i].reshape(C, N))
        m["grad"] = np.ascontiguousarray(grad[i].reshape(C, N))
        maps.append(m)
    return maps


def kernel(**inputs):
    from concourse import bacc
    from concourse.bass_utils import run_bass_kernel_spmd

    maps = _host_prep(inputs)
    if "nc" not in _COMPILED:
        nc = bacc.Bacc(None)
        _build(nc)
        nc.finalize()
        _COMPILED["nc"] = nc
    res = run_bass_kernel_spmd(_COMPILED["nc"], maps, core_ids=list(range(B)))
    return np.stack([np.asarray(res.results[i]["out"], np.float32).reshape(C, H, W)
                     for i in range(B)])
